# revision 5
# baseline (speedup 1.0000x reference)
"""StyleGAN-style modulated 3x3 conv on 8 Trainium2 NeuronCores.

Problem: y = conv2d(x, kernel * (style+1) / demod), SAME padding,
  x [B=8, H=128, W=128, C=256], kernel [3,3,C=256,F=256],
  style [B,1,1,C], demod[f] = sqrt(sum_{ky,kx,c} wmod^2 + 1e-8).

Sharding: data-parallel over batch B - each of the 8 cores convolves one
sample with its own modulated/demodulated kernel. No cross-core comm.

Algorithm (v2): 1-D Winograd F(2,3) along H. The 3 ky taps collapse into
4 transformed planes, so each output pair of rows needs 4x3(kx)x2(ch) =
24 matmul passes instead of the direct conv's 36: PE streamed columns
drop 1.5x (599k -> 399k per core, ~166us at 2.4 GHz).

  forward (DVE, bf16 2x): per tile-row i (output rows 2i, 2i+1), with
    d_a = xp[2i+a] (xp = zero-padded [C,130,130] image):
      V0 = d0-d2, V1 = d1+d2, V2 = d2-d1, V3 = d1-d3
  matmuls (PE): M_k[f,p] += sum_{kx,c} wt[k,kx,c,f] * V_k[c,p+kx-1],
    with the kx shift applied on the PSUM *output* AP (out cols 2-kx..)
    so the moving operand is one identical contiguous [128,390] AP per
    (k,ch) and tile seams land on junk pad columns.
  weights (DVE, head): wt0 = g0*s, wt1 = (g0+g1+g2)*s/2,
    wt2 = (g0-g1+g2)*s/2, wt3 = g2*s  (g = raw kernel taps, s = style+1),
    rounded to bf16.
  inverse (ACT+DVE, fused into PSUM drain, demod applied as per-partition
    scale): y_even = (M0+M1+M2)*invd, y_odd = (M1-M2-M3)*invd via
      s1 = ACT.copy(M1, scale=iv), s2 = ACT.copy(M2, scale=iv)
      a = STT(M0*iv + s1);  y_e = a + s2
      b = STT(M3*(-iv) + s1); y_o = b - s2
  demod invd computed as in the direct kernel: ACT squares of raw taps,
  GPSIMD accumulate, *(s+1)^2, channel-sum via matmul against ones.

Host does layout-only marshalling: shard over B, transpose+zero-pad x to
[ch,128,130,130] bf16, reorder kernel taps kx-major; gather interleaves
even/odd output row planes and strips pad columns.
"""

import sys
import os

for _p in ("/opt/trn_rl_repo", "/root/.axon_site", "/root/.axon_site/_ro/trn_rl_repo",
           "/root/.axon_site/_ro/pypackages"):
    if os.path.isdir(_p) and _p not in sys.path:
        sys.path.append(_p)

import numpy as np

B, H, W, C, F = 8, 128, 128, 256, 256
CH = C // 128                  # c-half count (contraction tiled by 128)
ROWS, COLS = H + 2, W + 2      # padded image dims
TR = H // 2                    # 64 Winograd tile-rows (2 output rows each)
CTR = 3                        # tile-rows per PSUM group (3*130=390 <= 512)
NCHUNK = (TR + CTR - 1) // CTR # 22 (21 full + 1 single-row)
N_CORES = 8

_COMPILED = {}


def _chunk_rows(c0):
    """tile-rows [r0, r1) of chunk c0."""
    r0 = CTR * c0
    r1 = min(TR, r0 + CTR)
    return r0, r1


def _build_nc():
    import concourse.bacc as bacc
    import concourse.mybir as mybir
    import concourse.tile as tile

    f32 = mybir.dt.float32
    bf16 = mybir.dt.bfloat16
    AF = mybir.ActivationFunctionType
    ALU = mybir.AluOpType

    nc = bacc.Bacc("TRN2", target_bir_lowering=False, debug=False,
                   num_devices=N_CORES)

    # x: zero-padded transposed image, bf16 (halves DMA bytes, full PE rate)
    xt_d = nc.dram_tensor("xt", [CH, 128, ROWS * COLS], bf16,
                          kind="ExternalInput").ap()
    st_d = nc.dram_tensor("st", [128, CH], f32, kind="ExternalInput").ap()
    # raw weights, tap axis kx-major (t = 3*kx + ky) so the G-combos for a
    # given kx only need that kx's 3-tap DMA chunk
    wk_d = nc.dram_tensor("wk", [CH, 128, 9, F], f32,
                          kind="ExternalInput").ap()
    # even/odd output row planes: [f_half, f, tile-row-major 64*130 px]
    ye_d = nc.dram_tensor("ye", [CH, 128, TR * COLS], f32,
                          kind="ExternalOutput").ap()
    yo_d = nc.dram_tensor("yo", [CH, 128, TR * COLS], f32,
                          kind="ExternalOutput").ap()

    with tile.TileContext(nc) as tc:
        with tc.tile_pool(name="pers", bufs=1) as pers, \
             tc.tile_pool(name="wtmp", bufs=3) as wtmp, \
             tc.tile_pool(name="vpool", bufs=3) as vpool, \
             tc.tile_pool(name="drain", bufs=4) as drain, \
             tc.tile_pool(name="psum", bufs=7, space="PSUM") as psum_pool, \
             tc.tile_pool(name="psumd", bufs=1, space="PSUM") as psum_d:

            # ---- style scalars: s = style+1, sh = s/2, nsh = -s/2 ----
            s_t = pers.tile([128, CH], f32, tag="s", name="s_t")
            nc.sync.dma_start(s_t[:], st_d)
            nc.vector.tensor_scalar_add(s_t[:], s_t[:], 1.0)
            sh_t = pers.tile([128, CH], f32, tag="sh", name="sh_t")
            nc.vector.tensor_scalar_mul(sh_t[:], s_t[:], 0.5)
            nsh_t = pers.tile([128, CH], f32, tag="nsh", name="nsh_t")
            nc.vector.tensor_scalar_mul(nsh_t[:], s_t[:], -0.5)
            s2_t = pers.tile([128, CH], f32, tag="s2", name="s2_t")
            nc.vector.tensor_mul(out=s2_t[:], in0=s_t[:], in1=s_t[:])
            ones_t = pers.tile([128, 1], f32, tag="ones", name="ones_t")
            nc.vector.memset(ones_t[:], 1.0)
            eps_t = pers.tile([128, 1], f32, tag="eps", name="eps_t")
            nc.vector.memset(eps_t[:], 1e-8)

            # ---- x image DMA, graded chunks (rows, per c-half) ----
            xt = [pers.tile([128, ROWS, COLS], bf16, tag=f"xt{ch}",
                            name=f"xt{ch}")
                  for ch in range(CH)]
            xbounds = [0, 8, 16, 32, 48, 74, 102, 130]
            for bi in range(len(xbounds) - 1):
                ra, rb = xbounds[bi], xbounds[bi + 1]
                for ch in range(CH):
                    nc.gpsimd.dma_start(
                        xt[ch][:, ra:rb, :],
                        xt_d[ch][:, ra * COLS:rb * COLS]
                        .rearrange("p (a b) -> p a b", a=rb - ra, b=COLS))

            # ---- raw weights + transformed/modulated weight tiles ----
            wraw = [pers.tile([128, 9, F], f32, tag=f"wraw{ch}",
                              name=f"wraw{ch}")
                    for ch in range(CH)]
            # wt[k][kx][ch]: bf16 [128, F] stationary tiles
            wt = [[[pers.tile([128, F], bf16, tag=f"wt{k}_{kx}_{ch}",
                              name=f"wt{k}_{kx}_{ch}")
                    for ch in range(CH)] for kx in range(3)]
                  for k in range(4)]
            sq = [[None] * 9 for _ in range(CH)]

            def emit_wk_chunk(kx):
                for ch in range(CH):
                    nc.sync.dma_start(wraw[ch][:, 3 * kx:3 * kx + 3],
                                      wk_d[ch][:, 3 * kx:3 * kx + 3])

            def emit_combos(kx):
                # DVE: transformed weights for this kx (both c-halves)
                for ch in range(CH):
                    g0 = wraw[ch][:, 3 * kx + 0]
                    g1 = wraw[ch][:, 3 * kx + 1]
                    g2 = wraw[ch][:, 3 * kx + 2]
                    sc = s_t[:, ch:ch + 1]
                    shc = sh_t[:, ch:ch + 1]
                    nshc = nsh_t[:, ch:ch + 1]
                    nc.vector.tensor_scalar_mul(wt[0][kx][ch][:], g0, sc)
                    nc.vector.tensor_scalar_mul(wt[3][kx][ch][:], g2, sc)
                    m2h = wtmp.tile([128, F], f32, tag="m2h", name="m2h")
                    nc.vector.tensor_scalar_mul(m2h[:], g2, shc)
                    uh = wtmp.tile([128, F], f32, tag="uh", name="uh")
                    nc.vector.scalar_tensor_tensor(
                        uh[:], g0, shc, m2h[:], op0=ALU.mult, op1=ALU.add)
                    nc.vector.scalar_tensor_tensor(
                        wt[1][kx][ch][:], g1, shc, uh[:],
                        op0=ALU.mult, op1=ALU.add)
                    nc.vector.scalar_tensor_tensor(
                        wt[2][kx][ch][:], g1, nshc, uh[:],
                        op0=ALU.mult, op1=ALU.add)

            def emit_squares(kx):
                # ACT: squares of raw taps (for demod), accumulated on GPSIMD
                for ch in range(CH):
                    for ky in range(3):
                        t = 3 * kx + ky
                        sqt = wtmp.tile([128, F], f32, tag=f"sq{ch}",
                                        name=f"sq{ch}_{t}", bufs=3)
                        nc.scalar.activation(sqt[:], wraw[ch][:, t],
                                             AF.Square)
                        sq[ch][t] = sqt

            acc = [pers.tile([128, F], f32, tag=f"acc{ch}", name=f"acc{ch}")
                   for ch in range(CH)]

            def emit_acc(kx):
                for ch in range(CH):
                    for ky in range(3):
                        t = 3 * kx + ky
                        if t == 0:
                            nc.gpsimd.tensor_copy(acc[ch][:], sq[ch][t][:])
                        else:
                            nc.gpsimd.tensor_add(acc[ch][:], acc[ch][:],
                                                 sq[ch][t][:])

            # ---- V planes (forward transform), chunk granularity ----
            vtiles = {}

            def emit_v(c0):
                r0, r1 = _chunk_rows(c0)
                nr = r1 - r0
                for ch in range(CH):
                    dall = [xt[ch][:, 2 * r0 + a: 2 * r0 + a + 2 * nr - 1:2, :]
                            for a in range(4)]
                    for k, (ia, ib_, op) in enumerate(
                            [(0, 2, "sub"), (1, 2, "add"),
                             (2, 1, "sub"), (1, 3, "sub")]):
                        v = vpool.tile([128, CTR, COLS], bf16,
                                       tag=f"v{k}_{ch}", name=f"v{k}_{ch}")
                        vv = v[:, :nr, :]
                        if op == "add":
                            nc.vector.tensor_add(vv, dall[ia], dall[ib_])
                        else:
                            nc.vector.tensor_sub(vv, dall[ia], dall[ib_])
                        vtiles[(c0, k, ch)] = v

            # ---- demod reciprocal (off critical path) ----
            iv = [None] * CH
            niv = [None] * CH

            def emit_invd():
                for ch in range(CH):
                    nc.gpsimd.tensor_scalar_mul(acc[ch][:], acc[ch][:],
                                                s2_t[:, ch:ch + 1])
                for fh in range(CH):
                    d2 = psum_d.tile([128, 1], f32, tag="d2",
                                     name=f"d2_{fh}")
                    for ch in range(CH):
                        nc.tensor.matmul(d2[:],
                                         acc[ch][:, fh * 128:(fh + 1) * 128],
                                         ones_t[:], start=(ch == 0),
                                         stop=(ch == CH - 1))
                    dm = pers.tile([128, 1], f32, tag=f"dm{fh}",
                                   name=f"dm{fh}")
                    nc.scalar.activation(dm[:], d2[:], AF.Sqrt, bias=eps_t[:])
                    ivt = pers.tile([128, 1], f32, tag=f"iv{fh}",
                                    name=f"iv{fh}")
                    nc.vector.reciprocal(ivt[:], dm[:])
                    nivt = pers.tile([128, 1], f32, tag=f"niv{fh}",
                                     name=f"niv{fh}")
                    nc.vector.tensor_scalar_mul(nivt[:], ivt[:], -1.0)
                    iv[fh] = ivt
                    niv[fh] = nivt

            # ---- head emission: weights + first V chunks interleaved ----
            emit_wk_chunk(0)
            emit_wk_chunk(1)
            emit_wk_chunk(2)
            emit_combos(0)
            emit_v(0)
            emit_combos(1)
            emit_v(1)
            emit_combos(2)
            emit_v(2)
            for kx in range(3):
                emit_squares(kx)
                emit_acc(kx)

            # ---- main loop: conv groups + fused inverse-transform drain ----
            invd_emitted = False
            for c0 in range(NCHUNK):
                r0, r1 = _chunk_rows(c0)
                nr = r1 - r0
                npx = nr * COLS           # 390 (or 130 for the last chunk)
                p0 = r0 * COLS
                for fh in range(CH):
                    pts = []
                    for k in range(4):
                        pt = psum_pool.tile([128, CTR * COLS + 2], f32,
                                            tag="pt", name=f"pt{k}")
                        for kx in range(3):
                            for ch in range(CH):
                                v = vtiles[(c0, k, ch)]
                                nc.tensor.matmul(
                                    pt[:, 2 - kx:2 - kx + npx],
                                    wt[k][kx][ch][:, fh * 128:(fh + 1) * 128],
                                    v[:, :nr, :].rearrange(
                                        "p a b -> p (a b)"),
                                    start=(kx == 0 and ch == 0),
                                    stop=(kx == 2 and ch == CH - 1))
                        pts.append(pt)
                    if not invd_emitted:
                        emit_invd()
                        invd_emitted = True
                    # drain: y_e = (M0+M1+M2)*iv, y_o = (M1-M2-M3)*iv
                    nv = npx - 2          # valid cols [2, npx)
                    s1 = drain.tile([128, CTR * COLS - 2], f32, tag="s1",
                                    name="s1")
                    nc.scalar.activation(s1[:, :nv], pts[1][:, 2:2 + nv],
                                         AF.Copy, scale=iv[fh][:])
                    s2c = drain.tile([128, CTR * COLS - 2], f32, tag="s2c",
                                     name="s2c")
                    nc.scalar.activation(s2c[:, :nv], pts[2][:, 2:2 + nv],
                                         AF.Copy, scale=iv[fh][:])
                    at = drain.tile([128, CTR * COLS - 2], f32, tag="at",
                                    name="at")
                    nc.vector.scalar_tensor_tensor(
                        at[:, :nv], pts[0][:, 2:2 + nv], iv[fh][:],
                        s1[:, :nv], op0=ALU.mult, op1=ALU.add)
                    yet = drain.tile([128, CTR * COLS - 2], f32, tag="ye",
                                     name="yet")
                    nc.vector.tensor_add(yet[:, :nv], at[:, :nv],
                                         s2c[:, :nv])
                    bt = drain.tile([128, CTR * COLS - 2], f32, tag="bt",
                                    name="bt")
                    nc.vector.scalar_tensor_tensor(
                        bt[:, :nv], pts[3][:, 2:2 + nv], niv[fh][:],
                        s1[:, :nv], op0=ALU.mult, op1=ALU.add)
                    yot = drain.tile([128, CTR * COLS - 2], f32, tag="yo",
                                     name="yot")
                    nc.vector.tensor_sub(yot[:, :nv], bt[:, :nv],
                                         s2c[:, :nv])
                    nc.gpsimd.dma_start(ye_d[fh][:, p0 + 1:p0 + 1 + nv],
                                        yet[:, :nv])
                    nc.gpsimd.dma_start(yo_d[fh][:, p0 + 1:p0 + 1 + nv],
                                        yot[:, :nv])
                # keep V production ~2 chunks ahead of the conv
                if c0 + 3 < NCHUNK:
                    emit_v(c0 + 3)

    nc.compile()
    return nc


def _get_nc():
    if "nc" not in _COMPILED:
        _COMPILED["nc"] = _build_nc()
    return _COMPILED["nc"]


def _prep_in_maps(x, style, kernel):
    """Host-side layout marshalling: shard over B, transpose+pad x."""
    x = np.ascontiguousarray(x, dtype=np.float32)
    style = np.ascontiguousarray(style, dtype=np.float32)
    kernel = np.ascontiguousarray(kernel, dtype=np.float32)
    # [3(ky),3(kx),C,F] -> [ch, c, t=3*kx+ky, f]
    wk = np.ascontiguousarray(
        kernel.reshape(3, 3, CH, 128, F).transpose(2, 3, 1, 0, 4)
        .reshape(CH, 128, 9, F))
    import ml_dtypes
    in_maps = []
    for b in range(B):
        xp = np.zeros((C, ROWS, COLS), dtype=ml_dtypes.bfloat16)
        xp[:, 1:H + 1, 1:W + 1] = x[b].transpose(2, 0, 1)
        xt = np.ascontiguousarray(xp.reshape(CH, 128, ROWS * COLS))
        st = np.ascontiguousarray(style[b].reshape(CH, 128).T)
        in_maps.append({"xt": xt, "st": st, "wk": wk})
    return in_maps


def run_cores(x, style, kernel, trace=False, trace_cores=None):
    """Compile (cached) + run on the 8 NeuronCores. Returns (y, results)."""
    from concourse.bass_utils import run_bass_kernel_spmd

    nc = _get_nc()
    in_maps = _prep_in_maps(x, style, kernel)
    kwargs = {}
    if trace:
        kwargs.update(trace=True, trace_cores=trace_cores)
    res = run_bass_kernel_spmd(nc, in_maps, list(range(N_CORES)), **kwargs)
    # ye/yo [fh, 128, 64*130]: rows 2i / 2i+1, pad cols stripped on host
    y = np.empty((B, H, W, F), dtype=np.float32)
    for b in range(B):
        for (nm, off) in (("ye", 0), ("yo", 1)):
            pl = res.results[b][nm].reshape(CH * 128, TR, COLS)[:, :, 1:W + 1]
            y[b, off::2] = pl.transpose(1, 2, 0)
    return y, res


def kernel(x, style, kernel):
    y, _ = run_cores(x, style, kernel)
    return y.astype(np.float32)


# revision 7
# speedup vs baseline: 1.0538x; 1.0538x over previous
"""StyleGAN-style modulated 3x3 conv on 8 Trainium2 NeuronCores.

Problem: y = conv2d(x, kernel * (style+1) / demod), SAME padding,
  x [B=8, H=128, W=128, C=256], kernel [3,3,C=256,F=256],
  style [B,1,1,C], demod[f] = sqrt(sum_{ky,kx,c} wmod^2 + 1e-8).

Sharding: data-parallel over batch B - each of the 8 cores convolves one
sample with its own modulated/demodulated kernel. No cross-core comm.

Algorithm: 1-D Winograd F(2,3) along H. The 3 ky taps collapse into 4
transformed planes, so each output row-pair needs 4x3(kx)x2(ch) = 24
matmul passes instead of the direct conv's 36: PE streamed columns drop
1.5x (599k -> 399k per core, ~166us at 2.4 GHz).

  forward (DVE, bf16 2x): per tile-row i (output rows 2i, 2i+1), with
    d_a = xp[2i+a] (xp = zero-padded [C,130,130] bf16 image):
      V0 = d0-d2, V1 = d1+d2, V2 = d2-d1, V3 = d1-d3
  matmuls (PE): M_k[f,p] += sum_{kx,c} wt[k,kx,c,f] * V_k[c,p+kx-1],
    kx shift applied on the PSUM *output* AP (out cols 2-kx..) so the
    moving operand is one contiguous [128,390] AP per (k,ch) and tile
    seams land on junk pad columns.
  weights (DVE bf16, head): wt0 = g0*s, wt1 = (g0+g1+g2)*s/2,
    wt2 = (g0-g1+g2)*s/2, wt3 = g2*s  (g = raw taps, s = style+1).
  demod (off critical path): sq = ACT.Square(wraw) one op per c-half;
    d2[f] = sum_{c,t} sq * s(c)^2 via 36 tiny matmuls with moving
    operand s^2 [128,1]; invd = 1/sqrt(d2+1e-8).
  inverse (ACT+DVE, fused into PSUM drain, demod as per-partition scale):
    y_even = (M0+M1+M2)*invd, y_odd = (M1-M2-M3)*invd via
      s1 = ACT.copy(M1, scale=iv), s2 = ACT.copy(M2, scale=iv)
      a = STT(M0*iv + s1);  y_e = a + s2
      b = STT(M3*(-iv) + s1); y_o = b - s2
    y_e/y_o written to halves of one tile -> single DMA per group.

Host does layout-only marshalling: shard over B, transpose+zero-pad x
to [ch,128,130,130] bf16, reorder kernel taps kx-major (bf16); gather
interleaves even/odd output row planes and strips pad columns.
"""

import sys
import os

for _p in ("/opt/trn_rl_repo", "/root/.axon_site", "/root/.axon_site/_ro/trn_rl_repo",
           "/root/.axon_site/_ro/pypackages"):
    if os.path.isdir(_p) and _p not in sys.path:
        sys.path.append(_p)

import numpy as np

B, H, W, C, F = 8, 128, 128, 256, 256
CH = C // 128                  # c-half count (contraction tiled by 128)
ROWS, COLS = H + 2, W + 2      # padded image dims
TR = H // 2                    # 64 Winograd tile-rows (2 output rows each)
CTR = 3                        # tile-rows per PSUM group (3*130=390 <= 512)
NCHUNK = (TR + CTR - 1) // CTR # 22 (21 full + 1 single-row)
N_CORES = 8

_COMPILED = {}


def _chunk_rows(c0):
    r0 = CTR * c0
    r1 = min(TR, r0 + CTR)
    return r0, r1


def _build_nc():
    import concourse.bacc as bacc
    import concourse.mybir as mybir
    import concourse.tile as tile

    f32 = mybir.dt.float32
    bf16 = mybir.dt.bfloat16
    AF = mybir.ActivationFunctionType
    ALU = mybir.AluOpType

    nc = bacc.Bacc("TRN2", target_bir_lowering=False, debug=False,
                   num_devices=N_CORES)

    xt_d = nc.dram_tensor("xt", [CH, 128, ROWS * COLS], bf16,
                          kind="ExternalInput").ap()
    st_d = nc.dram_tensor("st", [128, CH], f32, kind="ExternalInput").ap()
    # raw weights bf16, tap axis kx-major (t = 3*kx + ky)
    wk_d = nc.dram_tensor("wk", [CH, 128, 9, F], bf16,
                          kind="ExternalInput").ap()
    # merged even/odd output row planes [f_half, f, {even,odd}, 64*130]
    y2_d = nc.dram_tensor("y2", [CH, 128, 2, TR * COLS], f32,
                          kind="ExternalOutput").ap()

    with tile.TileContext(nc) as tc:
        with tc.tile_pool(name="pers", bufs=1) as pers, \
             tc.tile_pool(name="wtmp", bufs=3) as wtmp, \
             tc.tile_pool(name="vpool", bufs=3) as vpool, \
             tc.tile_pool(name="drain", bufs=4) as drain, \
             tc.tile_pool(name="psum", bufs=7, space="PSUM") as psum_pool, \
             tc.tile_pool(name="psumd", bufs=1, space="PSUM") as psum_d:

            # ---- style scalars: s = style+1, sh = s/2, nsh = -s/2 ----
            s_t = pers.tile([128, CH], f32, tag="s", name="s_t")
            nc.sync.dma_start(s_t[:], st_d)
            nc.vector.tensor_scalar_add(s_t[:], s_t[:], 1.0)
            sh_t = pers.tile([128, CH], f32, tag="sh", name="sh_t")
            nc.vector.tensor_scalar_mul(sh_t[:], s_t[:], 0.5)
            nsh_t = pers.tile([128, CH], f32, tag="nsh", name="nsh_t")
            nc.vector.tensor_scalar_mul(nsh_t[:], s_t[:], -0.5)
            s2_t = pers.tile([128, CH], f32, tag="s2", name="s2_t")
            nc.vector.tensor_mul(out=s2_t[:], in0=s_t[:], in1=s_t[:])
            eps_t = pers.tile([128, 1], f32, tag="eps", name="eps_t")
            nc.vector.memset(eps_t[:], 1e-8)

            # ---- weight DMA: split c-halves across sync/scalar queues ----
            wraw = [pers.tile([128, 9, F], bf16, tag=f"wraw{ch}",
                              name=f"wraw{ch}")
                    for ch in range(CH)]
            nc.sync.dma_start(wraw[0][:], wk_d[0])
            nc.sync.dma_start(wraw[1][:], wk_d[1])

            # ---- x image DMA, graded row chunks (gpsimd queue) ----
            xt = [pers.tile([128, ROWS, COLS], bf16, tag=f"xt{ch}",
                            name=f"xt{ch}")
                  for ch in range(CH)]
            xbounds = [0, 8, 16, 32, 48, 74, 102, 130]
            for bi in range(len(xbounds) - 1):
                ra, rb = xbounds[bi], xbounds[bi + 1]
                for ch in range(CH):
                    nc.gpsimd.dma_start(
                        xt[ch][:, ra:rb, :],
                        xt_d[ch][:, ra * COLS:rb * COLS]
                        .rearrange("p (a b) -> p a b", a=rb - ra, b=COLS))

            # wt[k][kx][ch]: bf16 [128, F] stationary tiles
            wt = [[[pers.tile([128, F], bf16, tag=f"wt{k}_{kx}_{ch}",
                              name=f"wt{k}_{kx}_{ch}")
                    for ch in range(CH)] for kx in range(3)]
                  for k in range(4)]

            def emit_combos(kx):
                # DVE: transformed modulated weights (bf16 fast modes)
                for ch in range(CH):
                    g0 = wraw[ch][:, 3 * kx + 0]
                    g1 = wraw[ch][:, 3 * kx + 1]
                    g2 = wraw[ch][:, 3 * kx + 2]
                    sc = s_t[:, ch:ch + 1]
                    shc = sh_t[:, ch:ch + 1]
                    nshc = nsh_t[:, ch:ch + 1]
                    nc.vector.tensor_scalar_mul(wt[0][kx][ch][:], g0, sc)
                    nc.vector.tensor_scalar_mul(wt[3][kx][ch][:], g2, sc)
                    m2h = wtmp.tile([128, F], bf16, tag="m2h", name="m2h")
                    nc.vector.tensor_scalar_mul(m2h[:], g2, shc)
                    uh = wtmp.tile([128, F], bf16, tag="uh", name="uh")
                    nc.vector.scalar_tensor_tensor(
                        uh[:], g0, shc, m2h[:], op0=ALU.mult, op1=ALU.add)
                    nc.vector.scalar_tensor_tensor(
                        wt[1][kx][ch][:], g1, shc, uh[:],
                        op0=ALU.mult, op1=ALU.add)
                    nc.vector.scalar_tensor_tensor(
                        wt[2][kx][ch][:], g1, nshc, uh[:],
                        op0=ALU.mult, op1=ALU.add)

            # ---- squares for demod: one ACT op per c-half ----
            sqall = [pers.tile([128, 9, F], f32, tag=f"sqall{ch}",
                               name=f"sqall{ch}")
                     for ch in range(CH)]

            def emit_squares():
                for ch in range(CH):
                    nc.scalar.activation(
                        sqall[ch][:].rearrange("p a b -> p (a b)"),
                        wraw[ch][:].rearrange("p a b -> p (a b)"),
                        AF.Square)

            # ---- V planes (forward transform), chunk granularity ----
            vtiles = {}

            def emit_v(c0):
                r0, r1 = _chunk_rows(c0)
                nr = r1 - r0
                for ch in range(CH):
                    dall = [xt[ch][:, 2 * r0 + a: 2 * r0 + a + 2 * nr - 1:2, :]
                            for a in range(4)]
                    for k, (ia, ib_, op) in enumerate(
                            [(0, 2, "sub"), (1, 2, "add"),
                             (2, 1, "sub"), (1, 3, "sub")]):
                        v = vpool.tile([128, CTR, COLS], bf16,
                                       tag=f"v{k}_{ch}", name=f"v{k}_{ch}")
                        vv = v[:, :nr, :]
                        if op == "add":
                            nc.vector.tensor_add(vv, dall[ia], dall[ib_])
                        else:
                            nc.vector.tensor_sub(vv, dall[ia], dall[ib_])
                        vtiles[(c0, k, ch)] = v

            # ---- demod reciprocal (36 tiny matmuls, off critical path) ----
            iv = [None] * CH
            niv = [None] * CH

            def emit_invd():
                for fh in range(CH):
                    d2 = psum_d.tile([128, 1], f32, tag="d2",
                                     name=f"d2_{fh}")
                    for ch in range(CH):
                        for t in range(9):
                            nc.tensor.matmul(
                                d2[:],
                                sqall[ch][:, t, fh * 128:(fh + 1) * 128],
                                s2_t[:, ch:ch + 1],
                                start=(ch == 0 and t == 0),
                                stop=(ch == CH - 1 and t == 8))
                    dm = pers.tile([128, 1], f32, tag=f"dm{fh}",
                                   name=f"dm{fh}")
                    nc.scalar.activation(dm[:], d2[:], AF.Sqrt, bias=eps_t[:])
                    ivt = pers.tile([128, 1], f32, tag=f"iv{fh}",
                                    name=f"iv{fh}")
                    nc.vector.reciprocal(ivt[:], dm[:])
                    nivt = pers.tile([128, 1], f32, tag=f"niv{fh}",
                                     name=f"niv{fh}")
                    nc.vector.tensor_scalar_mul(nivt[:], ivt[:], -1.0)
                    iv[fh] = ivt
                    niv[fh] = nivt

            # ---- head emission ----
            emit_v(0)
            emit_combos(0)
            emit_v(1)
            emit_combos(1)
            emit_combos(2)
            emit_squares()
            emit_v(2)

            # ---- main loop: conv groups + fused inverse-transform drain ----
            invd_emitted = False
            for c0 in range(NCHUNK):
                r0, r1 = _chunk_rows(c0)
                nr = r1 - r0
                npx = nr * COLS           # 390 (or 130 for the last chunk)
                p0 = r0 * COLS
                for fh in range(CH):
                    pts = []
                    for k in range(4):
                        pt = psum_pool.tile([128, CTR * COLS + 2], f32,
                                            tag="pt", name=f"pt{k}")
                        for kx in range(3):
                            for ch in range(CH):
                                v = vtiles[(c0, k, ch)]
                                nc.tensor.matmul(
                                    pt[:, 2 - kx:2 - kx + npx],
                                    wt[k][kx][ch][:, fh * 128:(fh + 1) * 128],
                                    v[:, :nr, :].rearrange(
                                        "p a b -> p (a b)"),
                                    start=(kx == 0 and ch == 0),
                                    stop=(kx == 2 and ch == CH - 1))
                        pts.append(pt)
                    if not invd_emitted:
                        emit_invd()
                        invd_emitted = True
                    # drain: y_e = (M0+M1+M2)*iv, y_o = (M1-M2-M3)*iv
                    nv = npx - 2          # valid cols [2, npx)
                    s1 = drain.tile([128, CTR * COLS - 2], f32, tag="s1",
                                    name="s1")
                    nc.scalar.activation(s1[:, :nv], pts[1][:, 2:2 + nv],
                                         AF.Copy, scale=iv[fh][:])
                    s2c = drain.tile([128, CTR * COLS - 2], f32, tag="s2c",
                                     name="s2c")
                    nc.scalar.activation(s2c[:, :nv], pts[2][:, 2:2 + nv],
                                         AF.Copy, scale=iv[fh][:])
                    at = drain.tile([128, CTR * COLS - 2], f32, tag="at",
                                    name="at")
                    nc.vector.scalar_tensor_tensor(
                        at[:, :nv], pts[0][:, 2:2 + nv], iv[fh][:],
                        s1[:, :nv], op0=ALU.mult, op1=ALU.add)
                    bt = drain.tile([128, CTR * COLS - 2], f32, tag="bt",
                                    name="bt")
                    nc.vector.scalar_tensor_tensor(
                        bt[:, :nv], pts[3][:, 2:2 + nv], niv[fh][:],
                        s1[:, :nv], op0=ALU.mult, op1=ALU.add)
                    y2t = drain.tile([128, 2, CTR * COLS - 2], f32,
                                     tag="y2t", name="y2t")
                    nc.vector.tensor_add(y2t[:, 0, :nv], at[:, :nv],
                                         s2c[:, :nv])
                    nc.vector.tensor_sub(y2t[:, 1, :nv], bt[:, :nv],
                                         s2c[:, :nv])
                    nc.gpsimd.dma_start(
                        y2_d[fh][:, :, p0 + 1:p0 + 1 + nv], y2t[:, :, :nv])
                # keep V production ~2 chunks ahead of the conv
                if c0 + 3 < NCHUNK:
                    emit_v(c0 + 3)

    nc.compile()
    return nc


def _get_nc():
    if "nc" not in _COMPILED:
        _COMPILED["nc"] = _build_nc()
    return _COMPILED["nc"]


def _prep_in_maps(x, style, kernel):
    """Host-side layout marshalling: shard over B, transpose+pad x."""
    import ml_dtypes
    x = np.ascontiguousarray(x, dtype=np.float32)
    style = np.ascontiguousarray(style, dtype=np.float32)
    kernel = np.ascontiguousarray(kernel, dtype=np.float32)
    # [3(ky),3(kx),C,F] -> [ch, c, t=3*kx+ky, f], bf16
    wk = np.ascontiguousarray(
        kernel.reshape(3, 3, CH, 128, F).transpose(2, 3, 1, 0, 4)
        .reshape(CH, 128, 9, F).astype(ml_dtypes.bfloat16))
    in_maps = []
    for b in range(B):
        xp = np.zeros((C, ROWS, COLS), dtype=ml_dtypes.bfloat16)
        xp[:, 1:H + 1, 1:W + 1] = x[b].transpose(2, 0, 1)
        xt = np.ascontiguousarray(xp.reshape(CH, 128, ROWS * COLS))
        st = np.ascontiguousarray(style[b].reshape(CH, 128).T)
        in_maps.append({"xt": xt, "st": st, "wk": wk})
    return in_maps


def run_cores(x, style, kernel, trace=False, trace_cores=None):
    """Compile (cached) + run on the 8 NeuronCores. Returns (y, results)."""
    from concourse.bass_utils import run_bass_kernel_spmd

    nc = _get_nc()
    in_maps = _prep_in_maps(x, style, kernel)
    kwargs = {}
    if trace:
        kwargs.update(trace=True, trace_cores=trace_cores)
    res = run_bass_kernel_spmd(nc, in_maps, list(range(N_CORES)), **kwargs)
    # y2 [fh, 128, {even,odd}, 64*130]: rows 2i / 2i+1, pad cols stripped
    y = np.empty((B, H, W, F), dtype=np.float32)
    for b in range(B):
        pl = res.results[b]["y2"].reshape(CH * 128, 2, TR, COLS)
        y[b, 0::2] = pl[:, 0, :, 1:W + 1].transpose(1, 2, 0)
        y[b, 1::2] = pl[:, 1, :, 1:W + 1].transpose(1, 2, 0)
    return y, res


def kernel(x, style, kernel):
    y, _ = run_cores(x, style, kernel)
    return y.astype(np.float32)


# revision 8
# speedup vs baseline: 1.1380x; 1.0800x over previous
"""StyleGAN-style modulated 3x3 conv on 8 Trainium2 NeuronCores.

Problem: y = conv2d(x, kernel * (style+1) / demod), SAME padding,
  x [B=8, H=128, W=128, C=256], kernel [3,3,C=256,F=256],
  style [B,1,1,C], demod[f] = sqrt(sum_{ky,kx,c} wmod^2 + 1e-8).

Sharding: data-parallel over batch B - each of the 8 cores convolves one
sample with its own modulated/demodulated kernel. No cross-core comm.

Algorithm: 1-D Winograd F(2,3) along H. The 3 ky taps collapse into 4
transformed planes, so each output row-pair needs 4x3(kx)x2(ch) = 24
matmul passes instead of the direct conv's 36: PE streamed columns drop
1.5x (599k -> 399k per core, ~166us at 2.4 GHz).

  forward (DVE, bf16 2x, two PSUM-chunks per op): per tile-row i
    (output rows 2i, 2i+1), with d_a = xp[2i+a] (xp = zero-padded
    [C,130,130] bf16 image):
      V0 = d0-d2, V1 = d1+d2, V2 = d2-d1, V3 = d1-d3
  matmuls (PE): M_k[f,p] += sum_{kx,c} wt[k,kx,c,f] * V_k[c,p+kx-1],
    kx shift applied on the PSUM *output* AP (out cols 2-kx..) so the
    moving operand is one contiguous [128,390] AP per (k,ch) and tile
    seams land on junk pad columns. Banks filled in order k=0,3,1,2 to
    match weight-combo readiness at the head.
  weights (DVE bf16, head, k-ordered): wt0 = g0*s, wt3 = g2*s,
    t = g0+g2, t2 = t+g1, t3 = t-g1, wt1 = t2*s/2, wt2 = t3*s/2.
  demod (off critical path): sq = ACT.Square(wraw, scale=s) one op per
    c-half (= wmod^2, bf16); d2[f] = sum_{c,t} sq via 36 tiny matmuls
    against a ones column; invd = 1/sqrt(d2+1e-8).
  inverse (ACT+DVE+GPSIMD, fused into PSUM drain, demod applied as
  per-partition scale):
    y_even = (M0+M1+M2)*invd, y_odd = (M1-M2-M3)*invd via
      s1 = ACT.copy(M1, scale=iv), s2 = ACT.copy(M2, scale=iv)
      a = DVE.STT(M0*iv + s1);  b = DVE.STT(M3*(-iv) + s1)
      y_e = GPSIMD.add(a, s2);  y_o = GPSIMD.sub(b, s2)
    y_e/y_o written to halves of one tile -> single DMA per group,
    alternating gpsimd/sync trigger queues.

Host does layout-only marshalling: shard over B, transpose+zero-pad x
to [ch,128,130,130] bf16, reorder kernel taps kx-major (bf16); gather
interleaves even/odd output row planes and strips pad columns.
"""

import sys
import os

for _p in ("/opt/trn_rl_repo", "/root/.axon_site", "/root/.axon_site/_ro/trn_rl_repo",
           "/root/.axon_site/_ro/pypackages"):
    if os.path.isdir(_p) and _p not in sys.path:
        sys.path.append(_p)

import numpy as np

B, H, W, C, F = 8, 128, 128, 256, 256
CH = C // 128                  # c-half count (contraction tiled by 128)
ROWS, COLS = H + 2, W + 2      # padded image dims
TR = H // 2                    # 64 Winograd tile-rows (2 output rows each)
CTR = 3                        # tile-rows per PSUM group (3*130=390 <= 512)
NCHUNK = (TR + CTR - 1) // CTR # 22 (21 full + 1 single-row)
VG = 2 * CTR                   # tile-rows per V-production op (2 chunks)
N_CORES = 8

_COMPILED = {}


def _chunk_rows(c0):
    r0 = CTR * c0
    r1 = min(TR, r0 + CTR)
    return r0, r1


def _build_nc():
    import concourse.bacc as bacc
    import concourse.mybir as mybir
    import concourse.tile as tile

    f32 = mybir.dt.float32
    bf16 = mybir.dt.bfloat16
    AF = mybir.ActivationFunctionType
    ALU = mybir.AluOpType

    nc = bacc.Bacc("TRN2", target_bir_lowering=False, debug=False,
                   num_devices=N_CORES)

    xt_d = nc.dram_tensor("xt", [CH, 128, ROWS * COLS], bf16,
                          kind="ExternalInput").ap()
    st_d = nc.dram_tensor("st", [128, CH], f32, kind="ExternalInput").ap()
    # raw weights bf16, tap axis kx-major (t = 3*kx + ky)
    wk_d = nc.dram_tensor("wk", [CH, 128, 9, F], bf16,
                          kind="ExternalInput").ap()
    # merged even/odd output row planes [f_half, f, {even,odd}, 64*130]
    y2_d = nc.dram_tensor("y2", [CH, 128, 2, TR * COLS], f32,
                          kind="ExternalOutput").ap()

    with tile.TileContext(nc) as tc:
        with tc.tile_pool(name="pers", bufs=1) as pers, \
             tc.tile_pool(name="wtmp", bufs=3) as wtmp, \
             tc.tile_pool(name="vpool", bufs=3) as vpool, \
             tc.tile_pool(name="drain", bufs=4) as drain, \
             tc.tile_pool(name="psum", bufs=7, space="PSUM") as psum_pool, \
             tc.tile_pool(name="psumd", bufs=1, space="PSUM") as psum_d:

            # ---- style scalars: s = style+1, sh = s/2 ----
            s_t = pers.tile([128, CH], f32, tag="s", name="s_t")
            nc.sync.dma_start(s_t[:], st_d)
            nc.vector.tensor_scalar_add(s_t[:], s_t[:], 1.0)
            sh_t = pers.tile([128, CH], f32, tag="sh", name="sh_t")
            nc.vector.tensor_scalar_mul(sh_t[:], s_t[:], 0.5)
            eps_t = pers.tile([128, 1], f32, tag="eps", name="eps_t")
            nc.vector.memset(eps_t[:], 1e-8)
            ones_b = pers.tile([128, 1], bf16, tag="onesb", name="ones_b")
            nc.vector.memset(ones_b[:], 1.0)

            # ---- weight DMA (sync queue, ahead of x triggers) ----
            wraw = [pers.tile([128, 9, F], bf16, tag=f"wraw{ch}",
                              name=f"wraw{ch}")
                    for ch in range(CH)]
            nc.sync.dma_start(wraw[0][:], wk_d[0])
            nc.sync.dma_start(wraw[1][:], wk_d[1])

            # ---- x image DMA, graded row chunks (sync queue) ----
            xt = [pers.tile([128, ROWS, COLS], bf16, tag=f"xt{ch}",
                            name=f"xt{ch}")
                  for ch in range(CH)]
            xbounds = [0, 13, 26, 39, 52, 78, 104, 130]
            for bi in range(len(xbounds) - 1):
                ra, rb = xbounds[bi], xbounds[bi + 1]
                for ch in range(CH):
                    nc.sync.dma_start(
                        xt[ch][:, ra:rb, :],
                        xt_d[ch][:, ra * COLS:rb * COLS]
                        .rearrange("p (a b) -> p a b", a=rb - ra, b=COLS))

            # wt[k][kx][ch]: bf16 [128, F] stationary tiles
            wt = [[[pers.tile([128, F], bf16, tag=f"wt{k}_{kx}_{ch}",
                              name=f"wt{k}_{kx}_{ch}")
                    for ch in range(CH)] for kx in range(3)]
                  for k in range(4)]
            # shared per-(kx,ch) tap sums for k=1/2 combos
            tsum = [[None] * CH for _ in range(3)]

            def emit_combos(k):
                # DVE, k-major so bank-k weights are ready in MM order
                for kx in range(3):
                    for ch in range(CH):
                        g0 = wraw[ch][:, 3 * kx + 0]
                        g1 = wraw[ch][:, 3 * kx + 1]
                        g2 = wraw[ch][:, 3 * kx + 2]
                        sc = s_t[:, ch:ch + 1]
                        shc = sh_t[:, ch:ch + 1]
                        if k == 0:
                            nc.vector.tensor_scalar_mul(
                                wt[0][kx][ch][:], g0, sc)
                        elif k == 3:
                            nc.vector.tensor_scalar_mul(
                                wt[3][kx][ch][:], g2, sc)
                        elif k == 1:
                            t = wtmp.tile([128, F], bf16, tag=f"t{kx}_{ch}",
                                          name=f"t{kx}_{ch}", bufs=1)
                            nc.vector.tensor_add(t[:], g0, g2)
                            tsum[kx][ch] = t
                            t2 = wtmp.tile([128, F], bf16, tag="t2",
                                           name="t2")
                            nc.vector.tensor_add(t2[:], t[:], g1)
                            nc.vector.tensor_scalar_mul(
                                wt[1][kx][ch][:], t2[:], shc)
                        else:  # k == 2
                            t3 = wtmp.tile([128, F], bf16, tag="t3",
                                           name="t3")
                            nc.vector.tensor_sub(t3[:], tsum[kx][ch][:], g1)
                            nc.vector.tensor_scalar_mul(
                                wt[2][kx][ch][:], t3[:], shc)

            # ---- squares for demod: ACT.Square(wraw * s) -> wmod^2 ----
            sqall = [pers.tile([128, 9, F], bf16, tag=f"sqall{ch}",
                               name=f"sqall{ch}")
                     for ch in range(CH)]

            def emit_squares():
                for ch in range(CH):
                    nc.scalar.activation(
                        sqall[ch][:].rearrange("p a b -> p (a b)"),
                        wraw[ch][:].rearrange("p a b -> p (a b)"),
                        AF.Square, scale=s_t[:, ch:ch + 1])

            # ---- V planes (forward transform), 2-chunk granularity ----
            vtiles = {}

            def emit_v(g):
                r0 = VG * g
                nr = min(TR, r0 + VG) - r0
                for ch in range(CH):
                    dall = [xt[ch][:, 2 * r0 + a: 2 * r0 + a + 2 * nr - 1:2, :]
                            for a in range(4)]
                    for k, (ia, ib_, op) in enumerate(
                            [(0, 2, "sub"), (1, 2, "add"),
                             (2, 1, "sub"), (1, 3, "sub")]):
                        v = vpool.tile([128, VG, COLS], bf16,
                                       tag=f"v{k}_{ch}", name=f"v{k}_{ch}")
                        vv = v[:, :nr, :]
                        if op == "add":
                            nc.vector.tensor_add(vv, dall[ia], dall[ib_])
                        else:
                            nc.vector.tensor_sub(vv, dall[ia], dall[ib_])
                        vtiles[(g, k, ch)] = v

            # ---- demod reciprocal (36 tiny matmuls, off critical path) ----
            iv = [None] * CH
            niv = [None] * CH

            def emit_invd():
                for fh in range(CH):
                    d2 = psum_d.tile([128, 1], f32, tag="d2",
                                     name=f"d2_{fh}")
                    for ch in range(CH):
                        for t in range(9):
                            nc.tensor.matmul(
                                d2[:],
                                sqall[ch][:, t, fh * 128:(fh + 1) * 128],
                                ones_b[:],
                                start=(ch == 0 and t == 0),
                                stop=(ch == CH - 1 and t == 8))
                    dm = pers.tile([128, 1], f32, tag=f"dm{fh}",
                                   name=f"dm{fh}")
                    nc.scalar.activation(dm[:], d2[:], AF.Sqrt, bias=eps_t[:])
                    ivt = pers.tile([128, 1], f32, tag=f"iv{fh}",
                                    name=f"iv{fh}")
                    nc.vector.reciprocal(ivt[:], dm[:])
                    nivt = pers.tile([128, 1], f32, tag=f"niv{fh}",
                                     name=f"niv{fh}")
                    nc.vector.tensor_scalar_mul(nivt[:], ivt[:], -1.0)
                    iv[fh] = ivt
                    niv[fh] = nivt

            # ---- head emission ----
            emit_v(0)
            emit_combos(0)
            emit_combos(3)
            emit_combos(1)
            emit_combos(2)
            emit_squares()
            emit_v(1)

            # ---- main loop: conv groups + fused inverse-transform drain ----
            invd_emitted = False
            for c0 in range(NCHUNK):
                r0, r1 = _chunk_rows(c0)
                nr = r1 - r0
                npx = nr * COLS           # 390 (or 130 for the last chunk)
                p0 = r0 * COLS
                g, sub = divmod(c0, 2)
                for fh in range(CH):
                    pts = [None] * 4
                    for k in (0, 3, 1, 2):
                        pt = psum_pool.tile([128, CTR * COLS + 2], f32,
                                            tag="pt", name=f"pt{k}")
                        for kx in range(3):
                            for ch in range(CH):
                                v = vtiles[(g, k, ch)]
                                nc.tensor.matmul(
                                    pt[:, 2 - kx:2 - kx + npx],
                                    wt[k][kx][ch][:, fh * 128:(fh + 1) * 128],
                                    v[:, CTR * sub:CTR * sub + nr, :]
                                    .rearrange("p a b -> p (a b)"),
                                    start=(kx == 0 and ch == 0),
                                    stop=(kx == 2 and ch == CH - 1))
                        pts[k] = pt
                    if not invd_emitted:
                        emit_invd()
                        invd_emitted = True
                    # drain: y_e = (M0+M1+M2)*iv, y_o = (M1-M2-M3)*iv
                    nv = npx - 2          # valid cols [2, npx)
                    s1 = drain.tile([128, CTR * COLS - 2], f32, tag="s1",
                                    name="s1")
                    nc.scalar.activation(s1[:, :nv], pts[1][:, 2:2 + nv],
                                         AF.Copy, scale=iv[fh][:])
                    s2c = drain.tile([128, CTR * COLS - 2], f32, tag="s2c",
                                     name="s2c")
                    nc.scalar.activation(s2c[:, :nv], pts[2][:, 2:2 + nv],
                                         AF.Copy, scale=iv[fh][:])
                    at = drain.tile([128, CTR * COLS - 2], f32, tag="at",
                                    name="at")
                    nc.vector.scalar_tensor_tensor(
                        at[:, :nv], pts[0][:, 2:2 + nv], iv[fh][:],
                        s1[:, :nv], op0=ALU.mult, op1=ALU.add)
                    bt = drain.tile([128, CTR * COLS - 2], f32, tag="bt",
                                    name="bt")
                    nc.vector.scalar_tensor_tensor(
                        bt[:, :nv], pts[3][:, 2:2 + nv], niv[fh][:],
                        s1[:, :nv], op0=ALU.mult, op1=ALU.add)
                    y2t = drain.tile([128, 2, CTR * COLS - 2], f32,
                                     tag="y2t", name="y2t")
                    nc.gpsimd.tensor_add(y2t[:, 0, :nv], at[:, :nv],
                                         s2c[:, :nv])
                    nc.gpsimd.tensor_sub(y2t[:, 1, :nv], bt[:, :nv],
                                         s2c[:, :nv])
                    dq = nc.gpsimd if (c0 + fh) % 2 == 0 else nc.sync
                    dq.dma_start(
                        y2_d[fh][:, :, p0 + 1:p0 + 1 + nv], y2t[:, :, :nv])
                # keep V production ~2 chunks ahead of the conv
                if sub == 1 and (g + 2) * VG < TR + VG and (g + 2) <= \
                        (TR - 1) // VG:
                    emit_v(g + 2)

    nc.compile()
    return nc


def _get_nc():
    if "nc" not in _COMPILED:
        _COMPILED["nc"] = _build_nc()
    return _COMPILED["nc"]


def _prep_in_maps(x, style, kernel):
    """Host-side layout marshalling: shard over B, transpose+pad x."""
    import ml_dtypes
    x = np.ascontiguousarray(x, dtype=np.float32)
    style = np.ascontiguousarray(style, dtype=np.float32)
    kernel = np.ascontiguousarray(kernel, dtype=np.float32)
    # [3(ky),3(kx),C,F] -> [ch, c, t=3*kx+ky, f], bf16
    wk = np.ascontiguousarray(
        kernel.reshape(3, 3, CH, 128, F).transpose(2, 3, 1, 0, 4)
        .reshape(CH, 128, 9, F).astype(ml_dtypes.bfloat16))
    in_maps = []
    for b in range(B):
        xp = np.zeros((C, ROWS, COLS), dtype=ml_dtypes.bfloat16)
        xp[:, 1:H + 1, 1:W + 1] = x[b].transpose(2, 0, 1)
        xt = np.ascontiguousarray(xp.reshape(CH, 128, ROWS * COLS))
        st = np.ascontiguousarray(style[b].reshape(CH, 128).T)
        in_maps.append({"xt": xt, "st": st, "wk": wk})
    return in_maps


def run_cores(x, style, kernel, trace=False, trace_cores=None):
    """Compile (cached) + run on the 8 NeuronCores. Returns (y, results)."""
    from concourse.bass_utils import run_bass_kernel_spmd

    nc = _get_nc()
    in_maps = _prep_in_maps(x, style, kernel)
    kwargs = {}
    if trace:
        kwargs.update(trace=True, trace_cores=trace_cores)
    res = run_bass_kernel_spmd(nc, in_maps, list(range(N_CORES)), **kwargs)
    # y2 [fh, 128, {even,odd}, 64*130]: rows 2i / 2i+1, pad cols stripped
    y = np.empty((B, H, W, F), dtype=np.float32)
    for b in range(B):
        pl = res.results[b]["y2"].reshape(CH * 128, 2, TR, COLS)
        y[b, 0::2] = pl[:, 0, :, 1:W + 1].transpose(1, 2, 0)
        y[b, 1::2] = pl[:, 1, :, 1:W + 1].transpose(1, 2, 0)
    return y, res


def kernel(x, style, kernel):
    y, _ = run_cores(x, style, kernel)
    return y.astype(np.float32)
